# revision 5
# baseline (speedup 1.0000x reference)
"""Trainium2 Bass kernel for nn_MultiHeadAttention_68959994904763.

Sharding (8 NeuronCores): 2-D tensor-parallel — batch (2) x head-groups (4).
Core c handles batch b = c // 4 and heads [4g, 4g+4) with g = c % 4.
Each core computes a partial output o_heads @ W_o for its 4 heads; the
host sums the 4 partials per batch and adds the (host-folded) bias
b_o_eff = b_v.flatten() @ W_o + b_o.

Per-core kernel (all matmuls in float32r: ~4x fp32 PE throughput,
~1.5e-4 matmul rel-err):
  1. x^T via PE transposes (E on partitions).
  2. q^T/k^T = (W_qk-pair)^T x^T + bias, per head-pair [128, S]; v via v^T
     then PE re-transpose into per-(head, s-chunk) v_aug [128, 65] tiles
     with a ones column (denominator trick).
  3. Scores transposed: s^T[k, q] = k^T.T @ q^T per (head, q-window of 512,
     k-chunk of 128), causal tiles only. Exp on ACT (no max subtraction:
     |score| <= ~3 for this problem's distribution). Diagonal-crossing
     tiles masked by precomputed 0/1 masks.
  4. o_aug^T[65, q] accumulated over k-chunks: lhsT = v_aug (ones column
     makes row 64 the softmax denominator). Normalize: reciprocal of row
     64, broadcast across partitions via a K=1 PE outer product, DVE mul.
  5. out = sum_heads o^T.T @ W_o rows, accumulated in PSUM over head pairs.
"""

import os
import sys
import types

import numpy as np

S, E, D = 2048, 1024, 64
P = 128
NQ = 512  # q-window (moving operand) size
SC = S // P  # 16 s-chunks
EC = E // P  # 8 e-chunks
QW = S // NQ  # 4 q-windows
N_CORES = 8


def _ensure_axon_hooks():
    """Provide antenv.axon_hooks (NTFF profile hook registry) if the image
    lacks it, and register the ctypes-based hook so trace=True works."""
    try:
        from antenv.axon_hooks import get_axon_ntff_profile_hook  # noqa: F401
        return
    except ImportError:
        pass
    import antenv

    mod = types.ModuleType("antenv.axon_hooks")
    _h = [None]
    mod.set_axon_ntff_profile_hook = lambda h: _h.__setitem__(0, h)
    mod.get_axon_ntff_profile_hook = lambda: _h[0]
    sys.modules["antenv.axon_hooks"] = mod
    antenv.axon_hooks = mod
    try:
        from trn_agent_boot.trn_boot import _ntff_profile_via_ctypes

        so_path = "/opt/axon/libaxon_pjrt.so"
        if os.path.exists(so_path):
            mod.set_axon_ntff_profile_hook(_ntff_profile_via_ctypes(so_path))
    except Exception:
        pass


def _build_program():
    import concourse.bass as bass  # noqa: F401
    import concourse.mybir as mybir
    import concourse.tile as tile
    from concourse import bacc
    import contextlib

    f32 = mybir.dt.float32
    f32r = mybir.dt.float32r

    nc = bacc.Bacc("TRN2", target_bir_lowering=False, debug=False)

    x_d = nc.dram_tensor("x", [S, E], f32r, kind="ExternalInput").ap()
    wq_d = nc.dram_tensor("wq", [2, EC, P, P], f32r, kind="ExternalInput").ap()
    wk_d = nc.dram_tensor("wk", [2, EC, P, P], f32r, kind="ExternalInput").ap()
    wv_d = nc.dram_tensor("wv", [2, EC, P, P], f32r, kind="ExternalInput").ap()
    bq_d = nc.dram_tensor("bq", [2, P, 1], f32, kind="ExternalInput").ap()
    bk_d = nc.dram_tensor("bk", [2, P, 1], f32, kind="ExternalInput").ap()
    wo_d = nc.dram_tensor("wo", [2, P, E], f32r, kind="ExternalInput").ap()
    mk_d = nc.dram_tensor("masks", [4, P, NQ], f32r, kind="ExternalInput").ap()
    id_d = nc.dram_tensor("ident", [P, P], f32r, kind="ExternalInput").ap()
    on_d = nc.dram_tensor("ones64", [1, D], f32r, kind="ExternalInput").ap()
    out_d = nc.dram_tensor("out", [S, E], f32, kind="ExternalOutput").ap()

    Act = mybir.ActivationFunctionType

    with tile.TileContext(nc) as tc:
        with contextlib.ExitStack() as top:
            persist = top.enter_context(tc.tile_pool(name="persist", bufs=1))

            # --- persistent constants / weights ---
            ident = persist.tile([P, P], f32r, tag="ident")
            nc.sync.dma_start(ident[:], id_d[:])
            ones64 = persist.tile([1, D], f32r, tag="ones64")
            nc.sync.dma_start(ones64[:], on_d[:])
            bq_t, bk_t = [], []
            for pr in range(2):
                t = persist.tile([P, 1], f32, tag=f"bq{pr}")
                nc.sync.dma_start(t[:], bq_d[pr])
                bq_t.append(t)
                t = persist.tile([P, 1], f32, tag=f"bk{pr}")
                nc.sync.dma_start(t[:], bk_d[pr])
                bk_t.append(t)
            wo_t = []
            for pr in range(2):
                t = persist.tile([P, E], f32r, tag=f"wo{pr}")
                nc.sync.dma_start(t[:], wo_d[pr])
                wo_t.append(t)
            mask_t = []
            for j in range(4):
                t = persist.tile([P, NQ], f32r, tag=f"mask{j}")
                nc.sync.dma_start(t[:], mk_d[j])
                mask_t.append(t)

            # persistent activations
            qT = [persist.tile([P, S], f32r, tag=f"qT{pr}", name=f"qT{pr}") for pr in range(2)]
            kT = [persist.tile([P, S], f32r, tag=f"kT{pr}", name=f"kT{pr}") for pr in range(2)]
            oT = [persist.tile([P, S], f32r, tag=f"oT{pr}", name=f"oT{pr}") for pr in range(2)]
            # v_aug per (head, s-chunk): [128, 65], col 64 = 1.0
            va = [
                [persist.tile([P, D + 1], f32r, tag=f"va{h}_{sc}", name=f"va{h}_{sc}") for sc in range(SC)]
                for h in range(4)
            ]

            # ---------- Phases 1+2: x^T, QKV ----------
            with contextlib.ExitStack() as ph12:
                wpool = ph12.enter_context(tc.tile_pool(name="wqkv", bufs=1))
                wq_t = [[None] * EC for _ in range(2)]
                wk_t = [[None] * EC for _ in range(2)]
                wv_t = [[None] * EC for _ in range(2)]
                for pr in range(2):
                    for ec in range(EC):
                        for nm, store, dram in (
                            ("q", wq_t, wq_d),
                            ("k", wk_t, wk_d),
                            ("v", wv_t, wv_d),
                        ):
                            t = wpool.tile([P, P], f32r, tag=f"w{nm}{pr}_{ec}", name=f"w{nm}{pr}_{ec}")
                            nc.sync.dma_start(t[:], dram[pr, ec])
                            store[pr][ec] = t

                xTp = ph12.enter_context(tc.tile_pool(name="xT", bufs=1))
                xT = [xTp.tile([P, S], f32r, tag=f"xT{ec}", name=f"xT{ec}") for ec in range(EC)]

                xin = ph12.enter_context(tc.tile_pool(name="xin", bufs=3))
                ps_t = ph12.enter_context(
                    tc.tile_pool(name="ps_t", bufs=4, space="PSUM")
                )
                for sc in range(SC):
                    xt = xin.tile([P, E], f32r, tag="xin")
                    nc.sync.dma_start(xt[:], x_d[sc * P : (sc + 1) * P, :])
                    for ec in range(EC):
                        pt = ps_t.tile([P, P], f32r, tag="ptr")
                        nc.tensor.transpose(
                            pt[:], xt[:, ec * P : (ec + 1) * P], ident[:]
                        )
                        nc.vector.tensor_copy(
                            xT[ec][:, sc * P : (sc + 1) * P], pt[:]
                        )

                ps_qk = ph12.enter_context(
                    tc.tile_pool(name="ps_qk", bufs=2, space="PSUM")
                )
                vtmp = ph12.enter_context(tc.tile_pool(name="vtmp", bufs=2))
                for pr in range(2):
                    for w_t, b_t, dst in (
                        (wq_t[pr], bq_t[pr], qT[pr]),
                        (wk_t[pr], bk_t[pr], kT[pr]),
                    ):
                        for sw in range(QW):
                            pq = ps_qk.tile([P, NQ], f32, tag="pqk")
                            for ec in range(EC):
                                nc.tensor.matmul(
                                    pq[:],
                                    w_t[ec][:],
                                    xT[ec][:, sw * NQ : (sw + 1) * NQ],
                                    start=(ec == 0),
                                    stop=(ec == EC - 1),
                                )
                            nc.scalar.activation(
                                dst[:, sw * NQ : (sw + 1) * NQ],
                                pq[:],
                                Act.Identity,
                                bias=b_t[:],
                            )
                    # v^T then re-transpose into v_aug natural tiles
                    for sw in range(QW):
                        pv = ps_qk.tile([P, NQ], f32, tag="pqk")
                        for ec in range(EC):
                            nc.tensor.matmul(
                                pv[:],
                                wv_t[pr][ec][:],
                                xT[ec][:, sw * NQ : (sw + 1) * NQ],
                                start=(ec == 0),
                                stop=(ec == EC - 1),
                            )
                        vt = vtmp.tile([P, NQ], f32r, tag="vtmp")
                        nc.vector.tensor_copy(vt[:], pv[:])
                        for i in range(NQ // P):
                            sc = sw * (NQ // P) + i
                            pvt = ps_t.tile([P, P], f32r, tag="ptr")
                            nc.tensor.transpose(
                                pvt[:], vt[:, i * P : (i + 1) * P], ident[:]
                            )
                            for hh in range(2):
                                h = pr * 2 + hh
                                nc.vector.tensor_copy(
                                    va[h][sc][:, 0:D],
                                    pvt[:, hh * D : (hh + 1) * D],
                                )
                                nc.vector.memset(
                                    va[h][sc][:, D : D + 1].bitcast(f32), 1.0
                                )

            # ---------- Phases 3+4: attention + W_o ----------
            with contextlib.ExitStack() as ph34:
                ps_s = ph34.enter_context(
                    tc.tile_pool(name="ps_s", bufs=3, space="PSUM")
                )
                ps_o = ph34.enter_context(
                    tc.tile_pool(name="ps_o", bufs=2, space="PSUM")
                )
                ps_b = ph34.enter_context(
                    tc.tile_pool(name="ps_b", bufs=1, space="PSUM")
                )
                ps_wo = ph34.enter_context(
                    tc.tile_pool(name="ps_wo", bufs=1, space="PSUM")
                )
                epool = ph34.enter_context(tc.tile_pool(name="epool", bufs=4))
                rpool = ph34.enter_context(tc.tile_pool(name="rpool", bufs=2))
                obuf = ph34.enter_context(tc.tile_pool(name="obuf", bufs=3))

                for qw in range(QW):
                    q_sl = slice(qw * NQ, (qw + 1) * NQ)
                    nkc = 4 * qw + 4  # causal k-chunks for this q-window
                    for h in range(4):
                        pr, off = h // 2, (h % 2) * D
                        po = ps_o.tile([D + 1, NQ], f32, tag="po")
                        es = [None] * nkc
                        pss = [None] * nkc
                        # software-pipelined: scores(kc+1) issued before o(kc)
                        for kc in range(nkc):
                            ps = ps_s.tile([P, NQ], f32, tag="pss")
                            nc.tensor.matmul(
                                ps[:],
                                kT[pr][off : off + D, kc * P : (kc + 1) * P],
                                qT[pr][off : off + D, q_sl],
                                start=True,
                                stop=True,
                            )
                            e = epool.tile([P, NQ], f32r, tag="e")
                            nc.scalar.activation(e[:], ps[:], Act.Exp)
                            j = kc - 4 * qw
                            if 0 <= j < 4:
                                nc.vector.tensor_mul(e[:], e[:], mask_t[j][:])
                            es[kc] = e
                            pss[kc] = ps
                            if kc > 0:
                                nc.tensor.matmul(
                                    po[:],
                                    va[h][kc - 1][:],
                                    es[kc - 1][:],
                                    start=(kc - 1 == 0),
                                    stop=False,
                                )
                        nc.tensor.matmul(
                            po[:],
                            va[h][nkc - 1][:],
                            es[nkc - 1][:],
                            start=(nkc - 1 == 0),
                            stop=True,
                        )
                        # normalize: recip of denominator row, PE broadcast
                        rc = rpool.tile([1, NQ], f32r, tag="rc")
                        with nc.allow_low_precision(reason="f32r reciprocal feeds PE broadcast"):
                            nc.vector.reciprocal(rc[:], po[D : D + 1, :])
                        pb = ps_b.tile([D, NQ], f32, tag="pb")
                        nc.tensor.matmul(
                            pb[:], ones64[:], rc[:], start=True, stop=True
                        )
                        rb = rpool.tile([D, NQ], f32, tag="rb")
                        nc.vector.tensor_copy(rb[:], pb[:])
                        nc.vector.tensor_mul(
                            oT[pr][off : off + D, q_sl], po[0:D, :], rb[:]
                        )

                    # W_o for the s-chunks of this q-window
                    for i in range(NQ // P):
                        sc = qw * (NQ // P) + i
                        pw = ps_wo.tile([P, E], f32, tag="pwo")
                        for pr in range(2):
                            for n in range(E // NQ):
                                nc.tensor.matmul(
                                    pw[:, n * NQ : (n + 1) * NQ],
                                    oT[pr][:, sc * P : (sc + 1) * P],
                                    wo_t[pr][:, n * NQ : (n + 1) * NQ],
                                    start=(pr == 0),
                                    stop=(pr == 1),
                                )
                        ob = obuf.tile([P, E], f32, tag="ob")
                        nc.vector.tensor_copy(ob[:], pw[:])
                        nc.sync.dma_start(out_d[sc * P : (sc + 1) * P, :], ob[:])

    nc.compile()
    return nc


def _host_shard(x, W_q, b_q, W_k, b_k, W_v, b_v, W_o, b_o):
    """Build the 8 per-core input maps. Returns (in_maps, b_o_eff)."""
    f32 = np.float32
    masks = np.zeros((4, P, NQ), dtype=f32)
    for j in range(4):
        for p in range(P):
            masks[j, p, j * P + p :] = 1.0
    ident = np.eye(P, dtype=f32)
    ones64 = np.ones((1, D), dtype=f32)

    in_maps = []
    for c in range(N_CORES):
        b, g = c // 4, c % 4
        heads = [4 * g + i for i in range(4)]
        wq = np.zeros((2, EC, P, P), dtype=f32)
        wk = np.zeros((2, EC, P, P), dtype=f32)
        wv = np.zeros((2, EC, P, P), dtype=f32)
        bq = np.zeros((2, P, 1), dtype=f32)
        bk = np.zeros((2, P, 1), dtype=f32)
        wo = np.zeros((2, P, E), dtype=f32)
        for pr in range(2):
            h0, h1 = heads[2 * pr], heads[2 * pr + 1]
            wpair_q = np.concatenate([W_q[h0], W_q[h1]], axis=1) * 0.125
            wpair_k = np.concatenate([W_k[h0], W_k[h1]], axis=1)
            wpair_v = np.concatenate([W_v[h0], W_v[h1]], axis=1)
            wq[pr] = wpair_q.reshape(EC, P, P)
            wk[pr] = wpair_k.reshape(EC, P, P)
            wv[pr] = wpair_v.reshape(EC, P, P)
            bq[pr, :, 0] = np.concatenate([b_q[h0], b_q[h1]]) * 0.125
            bk[pr, :, 0] = np.concatenate([b_k[h0], b_k[h1]])
            wo[pr] = W_o[h0 * D : h0 * D + 2 * D]
        in_maps.append(
            {
                "x": np.ascontiguousarray(x[b], dtype=f32),
                "wq": wq,
                "wk": wk,
                "wv": wv,
                "bq": bq,
                "bk": bk,
                "wo": wo,
                "masks": masks,
                "ident": ident,
                "ones64": ones64,
            }
        )
    b_o_eff = (b_v.reshape(-1).astype(f32) @ W_o.astype(f32) + b_o).astype(f32)
    return in_maps, b_o_eff


def _run(in_maps, trace=False):
    from concourse.bass_utils import run_bass_kernel_spmd

    nc = _build_program()
    return run_bass_kernel_spmd(
        nc, in_maps, core_ids=list(range(N_CORES)), trace=trace
    )


def kernel(x, W_q, b_q, W_k, b_k, W_v, b_v, W_o, b_o, _trace=False, _result_box=None):
    _ensure_axon_hooks()
    args = [np.asarray(a, dtype=np.float32) for a in (x, W_q, b_q, W_k, b_k, W_v, b_v, W_o, b_o)]
    in_maps, b_o_eff = _host_shard(*args)
    res = _run(in_maps, trace=_trace)
    if _result_box is not None:
        _result_box.append(res)
    B = x.shape[0]
    out = np.zeros((B, S, E), dtype=np.float32)
    for c in range(N_CORES):
        out[c // 4] += res.results[c]["out"]
    out += b_o_eff
    return out


# revision 6
# speedup vs baseline: 1.0947x; 1.0947x over previous
"""Trainium2 Bass kernel for nn_MultiHeadAttention_68959994904763.

Sharding (8 NeuronCores): 2-D tensor-parallel — batch (2) x head-groups (4).
Core c handles batch b = c // 4 and heads [4g, 4g+4) with g = c % 4.
Each core computes a partial output o_heads @ W_o for its 4 heads; the
host sums the 4 partials per batch and adds the (host-folded) bias
b_o_eff = b_v.flatten() @ W_o + b_o.

Per-core kernel (all matmuls in float32r: ~4x fp32 PE throughput,
~1.5e-4 matmul rel-err):
  1. x^T via PE transposes (E on partitions).
  2. q^T/k^T = (W_qk-pair)^T x^T + bias, per head-pair [128, S]; v via v^T
     then PE re-transpose into per-(head, s-chunk) v_aug [128, 65] tiles
     with a ones column (denominator trick).
  3. Scores transposed: s^T[k, q] = k^T.T @ q^T per (head, q-window of 512,
     k-chunk of 128), causal tiles only. Exp on ACT (no max subtraction:
     |score| <= ~3 for this problem's distribution). Diagonal-crossing
     tiles masked by precomputed 0/1 masks.
  4. o_aug^T[65, q] accumulated over k-chunks: lhsT = v_aug (ones column
     makes row 64 the softmax denominator). Normalize: reciprocal of row
     64, broadcast across partitions via a K=1 PE outer product, DVE mul.
  5. out = sum_heads o^T.T @ W_o rows, accumulated in PSUM over head pairs.
"""

import os
import sys
import types

import numpy as np

S, E, D = 2048, 1024, 64
P = 128
NQ = 512  # q-window (moving operand) size
SC = S // P  # 16 s-chunks
EC = E // P  # 8 e-chunks
QW = S // NQ  # 4 q-windows
N_CORES = 8


def _ensure_axon_hooks():
    """Provide antenv.axon_hooks (NTFF profile hook registry) if the image
    lacks it, and register the ctypes-based hook so trace=True works."""
    try:
        from antenv.axon_hooks import get_axon_ntff_profile_hook  # noqa: F401
        return
    except ImportError:
        pass
    import antenv

    mod = types.ModuleType("antenv.axon_hooks")
    _h = [None]
    mod.set_axon_ntff_profile_hook = lambda h: _h.__setitem__(0, h)
    mod.get_axon_ntff_profile_hook = lambda: _h[0]
    sys.modules["antenv.axon_hooks"] = mod
    antenv.axon_hooks = mod
    try:
        from trn_agent_boot.trn_boot import _ntff_profile_via_ctypes

        so_path = "/opt/axon/libaxon_pjrt.so"
        if os.path.exists(so_path):
            mod.set_axon_ntff_profile_hook(_ntff_profile_via_ctypes(so_path))
    except Exception:
        pass


def _build_program():
    import concourse.bass as bass  # noqa: F401
    import concourse.mybir as mybir
    import concourse.tile as tile
    from concourse import bacc
    import contextlib

    f32 = mybir.dt.float32
    f32r = mybir.dt.float32r

    nc = bacc.Bacc("TRN2", target_bir_lowering=False, debug=False)

    x_d = nc.dram_tensor("x", [S, E], f32r, kind="ExternalInput").ap()
    wq_d = nc.dram_tensor("wq", [2, EC, P, P], f32r, kind="ExternalInput").ap()
    wk_d = nc.dram_tensor("wk", [2, EC, P, P], f32r, kind="ExternalInput").ap()
    wv_d = nc.dram_tensor("wv", [2, EC, P, P], f32r, kind="ExternalInput").ap()
    bq_d = nc.dram_tensor("bq", [2, P, 1], f32, kind="ExternalInput").ap()
    bk_d = nc.dram_tensor("bk", [2, P, 1], f32, kind="ExternalInput").ap()
    wo_d = nc.dram_tensor("wo", [2, P, E], f32r, kind="ExternalInput").ap()
    mk_d = nc.dram_tensor("masks", [4, P, NQ], f32r, kind="ExternalInput").ap()
    id_d = nc.dram_tensor("ident", [P, P], f32r, kind="ExternalInput").ap()
    on_d = nc.dram_tensor("ones64", [1, D], f32r, kind="ExternalInput").ap()
    out_d = nc.dram_tensor("out", [S, E], f32, kind="ExternalOutput").ap()

    Act = mybir.ActivationFunctionType

    with tile.TileContext(nc) as tc:
        with contextlib.ExitStack() as top:
            persist = top.enter_context(tc.tile_pool(name="persist", bufs=1))

            # --- persistent constants / weights ---
            ident = persist.tile([P, P], f32r, tag="ident")
            nc.sync.dma_start(ident[:], id_d[:])
            ones64 = persist.tile([1, D], f32r, tag="ones64")
            nc.sync.dma_start(ones64[:], on_d[:])
            bq_t, bk_t = [], []
            for pr in range(2):
                t = persist.tile([P, 1], f32, tag=f"bq{pr}")
                nc.sync.dma_start(t[:], bq_d[pr])
                bq_t.append(t)
                t = persist.tile([P, 1], f32, tag=f"bk{pr}")
                nc.sync.dma_start(t[:], bk_d[pr])
                bk_t.append(t)
            wo_t = []
            for pr in range(2):
                t = persist.tile([P, E], f32r, tag=f"wo{pr}")
                nc.sync.dma_start(t[:], wo_d[pr])
                wo_t.append(t)
            mask_t = []
            for j in range(4):
                t = persist.tile([P, NQ], f32r, tag=f"mask{j}")
                nc.sync.dma_start(t[:], mk_d[j])
                mask_t.append(t)

            # persistent activations
            qT = [persist.tile([P, S], f32r, tag=f"qT{pr}", name=f"qT{pr}") for pr in range(2)]
            kT = [persist.tile([P, S], f32r, tag=f"kT{pr}", name=f"kT{pr}") for pr in range(2)]
            oT = [persist.tile([P, S], f32r, tag=f"oT{pr}", name=f"oT{pr}") for pr in range(2)]
            # v_aug per (head, s-chunk): [128, 65], col 64 = 1.0
            va = [
                [persist.tile([P, D + 1], f32r, tag=f"va{h}_{sc}", name=f"va{h}_{sc}") for sc in range(SC)]
                for h in range(4)
            ]

            # ---------- Phases 1+2: x^T, QKV ----------
            with contextlib.ExitStack() as ph12:
                wpool = ph12.enter_context(tc.tile_pool(name="wqkv", bufs=1))
                wq_t = [[None] * EC for _ in range(2)]
                wk_t = [[None] * EC for _ in range(2)]
                wv_t = [[None] * EC for _ in range(2)]
                for pr in range(2):
                    for ec in range(EC):
                        for nm, store, dram in (
                            ("q", wq_t, wq_d),
                            ("k", wk_t, wk_d),
                            ("v", wv_t, wv_d),
                        ):
                            t = wpool.tile([P, P], f32r, tag=f"w{nm}{pr}_{ec}", name=f"w{nm}{pr}_{ec}")
                            nc.sync.dma_start(t[:], dram[pr, ec])
                            store[pr][ec] = t

                xTp = ph12.enter_context(tc.tile_pool(name="xT", bufs=1))
                xT = [xTp.tile([P, S], f32r, tag=f"xT{ec}", name=f"xT{ec}") for ec in range(EC)]

                xin = ph12.enter_context(tc.tile_pool(name="xin", bufs=3))
                ps_t = ph12.enter_context(
                    tc.tile_pool(name="ps_t", bufs=4, space="PSUM")
                )
                for sc in range(SC):
                    xt = xin.tile([P, E], f32r, tag="xin")
                    nc.sync.dma_start(xt[:], x_d[sc * P : (sc + 1) * P, :])
                    for ec in range(EC):
                        pt = ps_t.tile([P, P], f32r, tag="ptr")
                        nc.tensor.transpose(
                            pt[:], xt[:, ec * P : (ec + 1) * P], ident[:]
                        )
                        nc.vector.tensor_copy(
                            xT[ec][:, sc * P : (sc + 1) * P], pt[:]
                        )

                ps_qk = ph12.enter_context(
                    tc.tile_pool(name="ps_qk", bufs=2, space="PSUM")
                )
                vtmp = ph12.enter_context(tc.tile_pool(name="vtmp", bufs=2))
                for pr in range(2):
                    for w_t, b_t, dst in (
                        (wq_t[pr], bq_t[pr], qT[pr]),
                        (wk_t[pr], bk_t[pr], kT[pr]),
                    ):
                        for sw in range(QW):
                            pq = ps_qk.tile([P, NQ], f32, tag="pqk")
                            for ec in range(EC):
                                nc.tensor.matmul(
                                    pq[:],
                                    w_t[ec][:],
                                    xT[ec][:, sw * NQ : (sw + 1) * NQ],
                                    start=(ec == 0),
                                    stop=(ec == EC - 1),
                                )
                            nc.scalar.activation(
                                dst[:, sw * NQ : (sw + 1) * NQ],
                                pq[:],
                                Act.Identity,
                                bias=b_t[:],
                            )
                    # v^T then re-transpose into v_aug natural tiles
                    for sw in range(QW):
                        pv = ps_qk.tile([P, NQ], f32, tag="pqk")
                        for ec in range(EC):
                            nc.tensor.matmul(
                                pv[:],
                                wv_t[pr][ec][:],
                                xT[ec][:, sw * NQ : (sw + 1) * NQ],
                                start=(ec == 0),
                                stop=(ec == EC - 1),
                            )
                        vt = vtmp.tile([P, NQ], f32r, tag="vtmp")
                        nc.vector.tensor_copy(vt[:], pv[:])
                        for i in range(NQ // P):
                            sc = sw * (NQ // P) + i
                            pvt = ps_t.tile([P, P], f32r, tag="ptr")
                            nc.tensor.transpose(
                                pvt[:], vt[:, i * P : (i + 1) * P], ident[:]
                            )
                            for hh in range(2):
                                h = pr * 2 + hh
                                nc.vector.tensor_copy(
                                    va[h][sc][:, 0:D],
                                    pvt[:, hh * D : (hh + 1) * D],
                                )
                                nc.vector.memset(
                                    va[h][sc][:, D : D + 1].bitcast(f32), 1.0
                                )

            # ---------- Phases 3+4: attention + W_o ----------
            with contextlib.ExitStack() as ph34:
                ps_s = ph34.enter_context(
                    tc.tile_pool(name="ps_s", bufs=4, space="PSUM")
                )
                ps_o = ph34.enter_context(
                    tc.tile_pool(name="ps_o", bufs=2, space="PSUM")
                )
                ps_wo = ph34.enter_context(
                    tc.tile_pool(name="ps_wo", bufs=1, space="PSUM")
                )
                epool = ph34.enter_context(tc.tile_pool(name="epool", bufs=4))
                rpool = ph34.enter_context(tc.tile_pool(name="rpool", bufs=2))
                obuf = ph34.enter_context(tc.tile_pool(name="obuf", bufs=3))

                def emit_wo(qw):
                    # out rows for this q-window's four s-chunks
                    for i in range(NQ // P):
                        sc = qw * (NQ // P) + i
                        pw = ps_wo.tile([P, E], f32, tag="pwo", name="pw")
                        for pr in range(2):
                            for n in range(E // NQ):
                                nc.tensor.matmul(
                                    pw[:, n * NQ : (n + 1) * NQ],
                                    oT[pr][:, sc * P : (sc + 1) * P],
                                    wo_t[pr][:, n * NQ : (n + 1) * NQ],
                                    start=(pr == 0),
                                    stop=(pr == 1),
                                )
                        ob = obuf.tile([P, E], f32, tag="ob", name="ob")
                        nc.vector.tensor_copy(ob[:], pw[:])
                        nc.sync.dma_start(out_d[sc * P : (sc + 1) * P, :], ob[:])

                pending_wo = None
                for qw in range(QW):
                    nkc = 4 * qw + 4  # causal k-chunks for this q-window
                    for h in range(4):
                        pr, off = h // 2, (h % 2) * D
                        po = ps_o.tile([D + 1, NQ], f32, tag="po", name="po")
                        es = [None] * nkc
                        sls = [None] * nkc

                        def emit_o(kc):
                            nc.tensor.matmul(
                                po[:, sls[kc]],
                                va[h][kc][:],
                                es[kc][:, sls[kc]],
                                start=(kc == 0),
                                stop=(kc == nkc - 1),
                                skip_group_check=True,
                            )

                        # software-pipelined: scores(kc+1) issued before o(kc)
                        for kc in range(nkc):
                            j = kc - 4 * qw
                            # diagonal tiles: only queries >= chunk start
                            qa = j * P if 0 < j < 4 else 0
                            sl = slice(qa, NQ)
                            sls[kc] = sl
                            ps = ps_s.tile([P, NQ], f32, tag="pss", name="ps")
                            nc.tensor.matmul(
                                ps[:, sl],
                                kT[pr][off : off + D, kc * P : (kc + 1) * P],
                                qT[pr][off : off + D, qw * NQ + qa : (qw + 1) * NQ],
                                start=True,
                                stop=True,
                                skip_group_check=True,
                            )
                            e = epool.tile([P, NQ], f32r, tag="e", name="e")
                            nc.scalar.activation(e[:, sl], ps[:, sl], Act.Exp)
                            if 0 <= j < 4:
                                nc.vector.tensor_mul(
                                    e[:, sl], e[:, sl], mask_t[j][:, sl]
                                )
                            es[kc] = e
                            if kc == 2 and pending_wo is not None:
                                emit_wo(pending_wo)
                                pending_wo = None
                            if kc > 0:
                                emit_o(kc - 1)
                        emit_o(nkc - 1)

                        # normalize off the PE: row copy -> POOL bcast ->
                        # DVE reciprocal -> DVE mul into oT
                        drow = rpool.tile([1, NQ], f32, tag="drow", name="drow")
                        nc.scalar.copy(drow[:], po[D : D + 1, :])
                        db = rpool.tile([D, NQ], f32, tag="db", name="db")
                        nc.gpsimd.partition_broadcast(db[:], drow[:])
                        rb = rpool.tile([D, NQ], f32, tag="rb", name="rb")
                        nc.vector.reciprocal(rb[:], db[:])
                        nc.vector.tensor_mul(
                            oT[pr][off : off + D, qw * NQ : (qw + 1) * NQ],
                            po[0:D, :],
                            rb[:],
                        )
                    pending_wo = qw
                emit_wo(pending_wo)

    nc.compile()
    return nc


def _host_shard(x, W_q, b_q, W_k, b_k, W_v, b_v, W_o, b_o):
    """Build the 8 per-core input maps. Returns (in_maps, b_o_eff)."""
    f32 = np.float32
    masks = np.zeros((4, P, NQ), dtype=f32)
    for j in range(4):
        for p in range(P):
            masks[j, p, j * P + p :] = 1.0
    ident = np.eye(P, dtype=f32)
    ones64 = np.ones((1, D), dtype=f32)

    in_maps = []
    for c in range(N_CORES):
        b, g = c // 4, c % 4
        heads = [4 * g + i for i in range(4)]
        wq = np.zeros((2, EC, P, P), dtype=f32)
        wk = np.zeros((2, EC, P, P), dtype=f32)
        wv = np.zeros((2, EC, P, P), dtype=f32)
        bq = np.zeros((2, P, 1), dtype=f32)
        bk = np.zeros((2, P, 1), dtype=f32)
        wo = np.zeros((2, P, E), dtype=f32)
        for pr in range(2):
            h0, h1 = heads[2 * pr], heads[2 * pr + 1]
            wpair_q = np.concatenate([W_q[h0], W_q[h1]], axis=1) * 0.125
            wpair_k = np.concatenate([W_k[h0], W_k[h1]], axis=1)
            wpair_v = np.concatenate([W_v[h0], W_v[h1]], axis=1)
            wq[pr] = wpair_q.reshape(EC, P, P)
            wk[pr] = wpair_k.reshape(EC, P, P)
            wv[pr] = wpair_v.reshape(EC, P, P)
            bq[pr, :, 0] = np.concatenate([b_q[h0], b_q[h1]]) * 0.125
            bk[pr, :, 0] = np.concatenate([b_k[h0], b_k[h1]])
            wo[pr] = W_o[h0 * D : h0 * D + 2 * D]
        in_maps.append(
            {
                "x": np.ascontiguousarray(x[b], dtype=f32),
                "wq": wq,
                "wk": wk,
                "wv": wv,
                "bq": bq,
                "bk": bk,
                "wo": wo,
                "masks": masks,
                "ident": ident,
                "ones64": ones64,
            }
        )
    b_o_eff = (b_v.reshape(-1).astype(f32) @ W_o.astype(f32) + b_o).astype(f32)
    return in_maps, b_o_eff


def _run(in_maps, trace=False):
    from concourse.bass_utils import run_bass_kernel_spmd

    nc = _build_program()
    return run_bass_kernel_spmd(
        nc, in_maps, core_ids=list(range(N_CORES)), trace=trace
    )


def kernel(x, W_q, b_q, W_k, b_k, W_v, b_v, W_o, b_o, _trace=False, _result_box=None):
    _ensure_axon_hooks()
    args = [np.asarray(a, dtype=np.float32) for a in (x, W_q, b_q, W_k, b_k, W_v, b_v, W_o, b_o)]
    in_maps, b_o_eff = _host_shard(*args)
    res = _run(in_maps, trace=_trace)
    if _result_box is not None:
        _result_box.append(res)
    B = x.shape[0]
    out = np.zeros((B, S, E), dtype=np.float32)
    for c in range(N_CORES):
        out[c // 4] += res.results[c]["out"]
    out += b_o_eff
    return out


# revision 7
# speedup vs baseline: 1.1455x; 1.0464x over previous
"""Trainium2 Bass kernel for nn_MultiHeadAttention_68959994904763.

Sharding (8 NeuronCores): 2-D tensor-parallel — batch (2) x head-groups (4).
Core c handles batch b = c // 4 and heads [4g, 4g+4) with g = c % 4.
Each core computes a partial output o_heads @ W_o for its 4 heads; the
host sums the 4 partials per batch and adds the (host-folded) bias
b_o_eff = b_v.flatten() @ W_o + b_o.

Per-core kernel (all matmuls in float32r: ~4x fp32 PE throughput,
~1.5e-4 matmul rel-err):
  1. x^T via PE transposes (E on partitions).
  2. q^T/k^T = (W_qk-pair)^T x^T + bias, per head-pair [128, S]; v via v^T
     then PE re-transpose into per-(head, s-chunk) v_aug [128, 65] tiles
     with a ones column (denominator trick).
  3. Scores transposed: s^T[k, q] = k^T.T @ q^T per (head, q-window of 512,
     k-chunk of 128), causal tiles only. Exp on ACT (no max subtraction:
     |score| <= ~3 for this problem's distribution). Diagonal-crossing
     tiles masked by precomputed 0/1 masks.
  4. o_aug^T[65, q] accumulated over k-chunks: lhsT = v_aug (ones column
     makes row 64 the softmax denominator). Normalize: reciprocal of row
     64, broadcast across partitions via a K=1 PE outer product, DVE mul.
  5. out = sum_heads o^T.T @ W_o rows, accumulated in PSUM over head pairs.
"""

import os
import sys
import types

import numpy as np

S, E, D = 2048, 1024, 64
P = 128
NQ = 512  # q-window (moving operand) size
SC = S // P  # 16 s-chunks
EC = E // P  # 8 e-chunks
QW = S // NQ  # 4 q-windows
N_CORES = 8


def _ensure_axon_hooks():
    """Provide antenv.axon_hooks (NTFF profile hook registry) if the image
    lacks it, and register the ctypes-based hook so trace=True works."""
    try:
        from antenv.axon_hooks import get_axon_ntff_profile_hook  # noqa: F401
        return
    except ImportError:
        pass
    import antenv

    mod = types.ModuleType("antenv.axon_hooks")
    _h = [None]
    mod.set_axon_ntff_profile_hook = lambda h: _h.__setitem__(0, h)
    mod.get_axon_ntff_profile_hook = lambda: _h[0]
    sys.modules["antenv.axon_hooks"] = mod
    antenv.axon_hooks = mod
    try:
        from trn_agent_boot.trn_boot import _ntff_profile_via_ctypes

        so_path = "/opt/axon/libaxon_pjrt.so"
        if os.path.exists(so_path):
            mod.set_axon_ntff_profile_hook(_ntff_profile_via_ctypes(so_path))
    except Exception:
        pass


def _build_program():
    import concourse.bass as bass  # noqa: F401
    import concourse.mybir as mybir
    import concourse.tile as tile
    from concourse import bacc
    import contextlib

    f32 = mybir.dt.float32
    f32r = mybir.dt.float32r
    bf16 = mybir.dt.bfloat16

    nc = bacc.Bacc("TRN2", target_bir_lowering=False, debug=False)

    x_d = nc.dram_tensor("x", [S, E], f32r, kind="ExternalInput").ap()
    wq_d = nc.dram_tensor("wq", [2, EC, P, P], f32r, kind="ExternalInput").ap()
    wk_d = nc.dram_tensor("wk", [2, EC, P, P], f32r, kind="ExternalInput").ap()
    wv_d = nc.dram_tensor("wv", [2, EC, P, P], f32r, kind="ExternalInput").ap()
    bq_d = nc.dram_tensor("bq", [2, P, 1], f32, kind="ExternalInput").ap()
    bk_d = nc.dram_tensor("bk", [2, P, 1], f32, kind="ExternalInput").ap()
    wo_d = nc.dram_tensor("wo", [2, P, E], f32r, kind="ExternalInput").ap()
    mk_d = nc.dram_tensor("masks", [4, P, NQ], bf16, kind="ExternalInput").ap()
    id_d = nc.dram_tensor("ident", [P, P], f32r, kind="ExternalInput").ap()
    on_d = nc.dram_tensor("ones64", [1, D], f32r, kind="ExternalInput").ap()
    out_d = nc.dram_tensor("out", [S, E], f32, kind="ExternalOutput").ap()

    Act = mybir.ActivationFunctionType

    with tile.TileContext(nc) as tc:
        with contextlib.ExitStack() as top:
            persist = top.enter_context(tc.tile_pool(name="persist", bufs=1))

            # --- persistent constants / weights ---
            ident = persist.tile([P, P], f32r, tag="ident")
            nc.sync.dma_start(ident[:], id_d[:])
            ones64 = persist.tile([1, D], f32r, tag="ones64")
            nc.sync.dma_start(ones64[:], on_d[:])
            bq_t, bk_t = [], []
            for pr in range(2):
                t = persist.tile([P, 1], f32, tag=f"bq{pr}")
                nc.sync.dma_start(t[:], bq_d[pr])
                bq_t.append(t)
                t = persist.tile([P, 1], f32, tag=f"bk{pr}")
                nc.sync.dma_start(t[:], bk_d[pr])
                bk_t.append(t)
            wo_t = []
            for pr in range(2):
                t = persist.tile([P, E], f32r, tag=f"wo{pr}")
                nc.sync.dma_start(t[:], wo_d[pr])
                wo_t.append(t)
            mask_t = []
            for j in range(4):
                t = persist.tile([P, NQ], bf16, tag=f"mask{j}")
                nc.sync.dma_start(t[:], mk_d[j])
                mask_t.append(t)

            # persistent activations
            qT = [persist.tile([P, S], bf16, tag=f"qT{pr}", name=f"qT{pr}") for pr in range(2)]
            kT = [persist.tile([P, S], bf16, tag=f"kT{pr}", name=f"kT{pr}") for pr in range(2)]
            oT = [persist.tile([P, S], f32r, tag=f"oT{pr}", name=f"oT{pr}") for pr in range(2)]
            # v_aug per (head, s-chunk): [128, 65], col 64 = 1.0
            va = [
                [persist.tile([P, D + 1], bf16, tag=f"va{h}_{sc}", name=f"va{h}_{sc}") for sc in range(SC)]
                for h in range(4)
            ]

            # ---------- Phases 1+2: x^T, QKV ----------
            with contextlib.ExitStack() as ph12:
                wpool = ph12.enter_context(tc.tile_pool(name="wqkv", bufs=1))
                wq_t = [[None] * EC for _ in range(2)]
                wk_t = [[None] * EC for _ in range(2)]
                wv_t = [[None] * EC for _ in range(2)]
                for pr in range(2):
                    for ec in range(EC):
                        for nm, store, dram in (
                            ("q", wq_t, wq_d),
                            ("k", wk_t, wk_d),
                            ("v", wv_t, wv_d),
                        ):
                            t = wpool.tile([P, P], f32r, tag=f"w{nm}{pr}_{ec}", name=f"w{nm}{pr}_{ec}")
                            nc.sync.dma_start(t[:], dram[pr, ec])
                            store[pr][ec] = t

                xTp = ph12.enter_context(tc.tile_pool(name="xT", bufs=1))
                xT = [xTp.tile([P, S], f32r, tag=f"xT{ec}", name=f"xT{ec}") for ec in range(EC)]

                xin = ph12.enter_context(tc.tile_pool(name="xin", bufs=3))
                ps_t = ph12.enter_context(
                    tc.tile_pool(name="ps_t", bufs=4, space="PSUM")
                )
                for sc in range(SC):
                    xt = xin.tile([P, E], f32r, tag="xin")
                    nc.sync.dma_start(xt[:], x_d[sc * P : (sc + 1) * P, :])
                    for ec in range(EC):
                        pt = ps_t.tile([P, P], f32r, tag="ptr")
                        nc.tensor.transpose(
                            pt[:], xt[:, ec * P : (ec + 1) * P], ident[:]
                        )
                        nc.vector.tensor_copy(
                            xT[ec][:, sc * P : (sc + 1) * P], pt[:]
                        )

                ps_qk = ph12.enter_context(
                    tc.tile_pool(name="ps_qk", bufs=2, space="PSUM")
                )
                vtmp = ph12.enter_context(tc.tile_pool(name="vtmp", bufs=2))
                for pr in range(2):
                    for w_t, b_t, dst in (
                        (wq_t[pr], bq_t[pr], qT[pr]),
                        (wk_t[pr], bk_t[pr], kT[pr]),
                    ):
                        for sw in range(QW):
                            pq = ps_qk.tile([P, NQ], f32, tag="pqk")
                            for ec in range(EC):
                                nc.tensor.matmul(
                                    pq[:],
                                    w_t[ec][:],
                                    xT[ec][:, sw * NQ : (sw + 1) * NQ],
                                    start=(ec == 0),
                                    stop=(ec == EC - 1),
                                )
                            nc.scalar.activation(
                                dst[:, sw * NQ : (sw + 1) * NQ],
                                pq[:],
                                Act.Identity,
                                bias=b_t[:],
                            )
                    # v^T then re-transpose into v_aug natural tiles
                    for sw in range(QW):
                        pv = ps_qk.tile([P, NQ], f32, tag="pqk")
                        for ec in range(EC):
                            nc.tensor.matmul(
                                pv[:],
                                wv_t[pr][ec][:],
                                xT[ec][:, sw * NQ : (sw + 1) * NQ],
                                start=(ec == 0),
                                stop=(ec == EC - 1),
                            )
                        vt = vtmp.tile([P, NQ], f32r, tag="vtmp")
                        nc.vector.tensor_copy(vt[:], pv[:])
                        for i in range(NQ // P):
                            sc = sw * (NQ // P) + i
                            pvt = ps_t.tile([P, P], f32r, tag="ptr")
                            nc.tensor.transpose(
                                pvt[:], vt[:, i * P : (i + 1) * P], ident[:]
                            )
                            for hh in range(2):
                                h = pr * 2 + hh
                                nc.vector.tensor_copy(
                                    va[h][sc][:, 0:D],
                                    pvt[:, hh * D : (hh + 1) * D],
                                )
                                nc.vector.memset(
                                    va[h][sc][:, D : D + 1], 1.0
                                )

            # ---------- Phases 3+4: attention + W_o ----------
            with contextlib.ExitStack() as ph34:
                ps_s = ph34.enter_context(
                    tc.tile_pool(name="ps_s", bufs=4, space="PSUM")
                )
                ps_o = ph34.enter_context(
                    tc.tile_pool(name="ps_o", bufs=2, space="PSUM")
                )
                ps_wo = ph34.enter_context(
                    tc.tile_pool(name="ps_wo", bufs=1, space="PSUM")
                )
                epool = ph34.enter_context(tc.tile_pool(name="epool", bufs=4))
                rpool = ph34.enter_context(tc.tile_pool(name="rpool", bufs=2))
                obuf = ph34.enter_context(tc.tile_pool(name="obuf", bufs=3))

                def emit_wo(qw):
                    # out rows for this q-window's four s-chunks
                    for i in range(NQ // P):
                        sc = qw * (NQ // P) + i
                        pw = ps_wo.tile([P, E], f32, tag="pwo", name="pw")
                        for pr in range(2):
                            for n in range(E // NQ):
                                nc.tensor.matmul(
                                    pw[:, n * NQ : (n + 1) * NQ],
                                    oT[pr][:, sc * P : (sc + 1) * P],
                                    wo_t[pr][:, n * NQ : (n + 1) * NQ],
                                    start=(pr == 0),
                                    stop=(pr == 1),
                                )
                        ob = obuf.tile([P, E], f32, tag="ob", name="ob")
                        nc.vector.tensor_copy(ob[:], pw[:])
                        nc.sync.dma_start(out_d[sc * P : (sc + 1) * P, :], ob[:])

                pending_wo = None
                for qw in range(QW):
                    nkc = 4 * qw + 4  # causal k-chunks for this q-window
                    for h in range(4):
                        pr, off = h // 2, (h % 2) * D
                        po = ps_o.tile([D + 1, NQ], f32, tag="po", name="po")
                        es = [None] * nkc
                        sls = [None] * nkc

                        def emit_o(kc):
                            nc.tensor.matmul(
                                po[:, sls[kc]],
                                va[h][kc][:],
                                es[kc][:, sls[kc]],
                                start=(kc == 0),
                                stop=(kc == nkc - 1),
                                skip_group_check=True,
                            )

                        # software-pipelined: scores(kc+1) issued before o(kc)
                        for kc in range(nkc):
                            j = kc - 4 * qw
                            # diagonal tiles: only queries >= chunk start
                            qa = j * P if 0 < j < 4 else 0
                            sl = slice(qa, NQ)
                            sls[kc] = sl
                            ps = ps_s.tile([P, NQ], f32, tag="pss", name="ps")
                            nc.tensor.matmul(
                                ps[:, sl],
                                kT[pr][off : off + D, kc * P : (kc + 1) * P],
                                qT[pr][off : off + D, qw * NQ + qa : (qw + 1) * NQ],
                                start=True,
                                stop=True,
                                skip_group_check=True,
                            )
                            e = epool.tile([P, NQ], bf16, tag="e", name="e")
                            nc.scalar.activation(e[:, sl], ps[:, sl], Act.Exp)
                            if 0 <= j < 4:
                                nc.vector.tensor_mul(
                                    e[:, sl], e[:, sl], mask_t[j][:, sl]
                                )
                            es[kc] = e
                            if kc == 2 and pending_wo is not None:
                                emit_wo(pending_wo)
                                pending_wo = None
                            if kc > 0:
                                emit_o(kc - 1)
                        emit_o(nkc - 1)

                        # normalize off the PE: row copy -> POOL bcast ->
                        # DVE reciprocal -> DVE mul into oT
                        drow = rpool.tile([1, NQ], f32, tag="drow", name="drow")
                        nc.scalar.copy(drow[:], po[D : D + 1, :])
                        db = rpool.tile([D, NQ], f32, tag="db", name="db")
                        nc.gpsimd.partition_broadcast(db[:], drow[:])
                        rb = rpool.tile([D, NQ], f32, tag="rb", name="rb")
                        nc.vector.reciprocal(rb[:], db[:])
                        nc.vector.tensor_mul(
                            oT[pr][off : off + D, qw * NQ : (qw + 1) * NQ],
                            po[0:D, :],
                            rb[:],
                        )
                    pending_wo = qw
                emit_wo(pending_wo)

    nc.compile()
    return nc


def _host_shard(x, W_q, b_q, W_k, b_k, W_v, b_v, W_o, b_o):
    """Build the 8 per-core input maps. Returns (in_maps, b_o_eff)."""
    import ml_dtypes

    f32 = np.float32
    masks = np.zeros((4, P, NQ), dtype=f32)
    for j in range(4):
        for p in range(P):
            masks[j, p, j * P + p :] = 1.0
    ident = np.eye(P, dtype=f32)
    ones64 = np.ones((1, D), dtype=f32)

    in_maps = []
    for c in range(N_CORES):
        b, g = c // 4, c % 4
        heads = [4 * g + i for i in range(4)]
        wq = np.zeros((2, EC, P, P), dtype=f32)
        wk = np.zeros((2, EC, P, P), dtype=f32)
        wv = np.zeros((2, EC, P, P), dtype=f32)
        bq = np.zeros((2, P, 1), dtype=f32)
        bk = np.zeros((2, P, 1), dtype=f32)
        wo = np.zeros((2, P, E), dtype=f32)
        for pr in range(2):
            h0, h1 = heads[2 * pr], heads[2 * pr + 1]
            wpair_q = np.concatenate([W_q[h0], W_q[h1]], axis=1) * 0.125
            wpair_k = np.concatenate([W_k[h0], W_k[h1]], axis=1)
            wpair_v = np.concatenate([W_v[h0], W_v[h1]], axis=1)
            wq[pr] = wpair_q.reshape(EC, P, P)
            wk[pr] = wpair_k.reshape(EC, P, P)
            wv[pr] = wpair_v.reshape(EC, P, P)
            bq[pr, :, 0] = np.concatenate([b_q[h0], b_q[h1]]) * 0.125
            bk[pr, :, 0] = np.concatenate([b_k[h0], b_k[h1]])
            wo[pr] = W_o[h0 * D : h0 * D + 2 * D]
        in_maps.append(
            {
                "x": np.ascontiguousarray(x[b], dtype=f32),
                "wq": wq,
                "wk": wk,
                "wv": wv,
                "bq": bq,
                "bk": bk,
                "wo": wo,
                "masks": masks.astype(ml_dtypes.bfloat16),
                "ident": ident,
                "ones64": ones64,
            }
        )
    b_o_eff = (b_v.reshape(-1).astype(f32) @ W_o.astype(f32) + b_o).astype(f32)
    return in_maps, b_o_eff


def _run(in_maps, trace=False):
    from concourse.bass_utils import run_bass_kernel_spmd

    nc = _build_program()
    return run_bass_kernel_spmd(
        nc, in_maps, core_ids=list(range(N_CORES)), trace=trace
    )


def kernel(x, W_q, b_q, W_k, b_k, W_v, b_v, W_o, b_o, _trace=False, _result_box=None):
    _ensure_axon_hooks()
    args = [np.asarray(a, dtype=np.float32) for a in (x, W_q, b_q, W_k, b_k, W_v, b_v, W_o, b_o)]
    in_maps, b_o_eff = _host_shard(*args)
    res = _run(in_maps, trace=_trace)
    if _result_box is not None:
        _result_box.append(res)
    B = x.shape[0]
    out = np.zeros((B, S, E), dtype=np.float32)
    for c in range(N_CORES):
        out[c // 4] += res.results[c]["out"]
    out += b_o_eff
    return out
